# revision 15
# baseline (speedup 1.0000x reference)
"""Trainium2 Bass kernel for nn_Encoder (2-branch, 2-layer GATv2 encoder).

Strategy (validated against reference by numpy prototype, rel-l2 ~8e-7):
- 8-way node sharding: core c owns dst nodes [2500c, 2500c+2500); edges
  partitioned by dst owner, sorted by dst, grouped per 128-dst tile, chunked
  into groups of 128 edges.
- att folded into projection weights on host via the sign-split identity
    att>=0: att*LR_.2(z) = 0.2*y + 0.8*relu(y),   y = att*z
    att<0 : att*LR_.2(z) = 5*y - 4*relu(y),       y = 0.2*att*z
  The linear (y) term becomes extra "S" weight columns; relu terms are
  computed per edge with ACT relu(0.8*t)/relu(4*t) and a segmented reduce.
- Per-edge gather via dma_gather; per-chunk xr broadcast and scatter-add
  aggregation via one-hot indicator matmuls on the tensor engine (fp32 PSUM).
- Softmax without max-subtraction (scores are O(10); exact in fp32).
- Aggregation runs in att-scaled feature space; unscaled (1/g) at the end.
- xl all-gathered across the 8 cores (AllGather collective) per layer.
"""

import sys

sys.path.insert(0, "/opt/trn_rl_repo")

import numpy as np

N, IN_DIM, HID, HEADS = 20000, 3000, 128, 4
S_DIM, P_DIM = 32, 32
NEG_SLOPE = 0.2
ETA = 1e-6
NCORES = 8
NLOC = N // NCORES  # 2500
P = 128
NT = (NLOC + P - 1) // P  # 20 tiles, last has 68 rows
KPAD1 = 3072  # IN_DIM padded
K1 = KPAD1 // P  # 24
MASK_NEG = -30000.0


# ---------------------------------------------------------------- host prep


def _fold_weights(Wl, Wr, att, Cseg):
    """Returns Wl_aug, Wr_aug [Din, F], pos_lists, neg_lists, g (per orig col).

    Feature layout: [H*Cseg pos blocks | H Sp cols | H*Cseg neg | H Sn cols],
    F = 2*H*Cseg + 2*H.  S cols are pre-scaled: Sp=0.2*sum(pos w), Sn holds
    NEGATED 5*sum(neg w) so that e = (Rp' + Sp) - (Rn' + Sn') works with
    Sn' = -5*sum(neg w) and the device's single subtract.
    """
    Din = Wl.shape[0]
    H, C = att.shape
    att_f = att.reshape(-1).astype(np.float64)
    g = np.where(att_f >= 0, att_f, NEG_SLOPE * att_f)
    Wl64, Wr64 = Wl.astype(np.float64), Wr.astype(np.float64)
    pos_lists, neg_lists = [], []
    for h in range(H):
        cols = np.arange(h * C, (h + 1) * C)
        s = att_f[cols] >= 0
        pos_lists.append(cols[s])
        neg_lists.append(cols[~s])
    F = 2 * H * Cseg + 2 * H
    Wla = np.zeros((Din, F), np.float64)
    Wra = np.zeros((Din, F), np.float64)
    for h in range(H):
        pl, nl = pos_lists[h], neg_lists[h]
        assert len(pl) <= Cseg and len(nl) <= Cseg
        Wla[:, h * Cseg : h * Cseg + len(pl)] = g[pl] * Wl64[:, pl]
        Wra[:, h * Cseg : h * Cseg + len(pl)] = g[pl] * Wr64[:, pl]
        off = H * Cseg + H + h * Cseg
        Wla[:, off : off + len(nl)] = g[nl] * Wl64[:, nl]
        Wra[:, off : off + len(nl)] = g[nl] * Wr64[:, nl]
        # S columns (linear terms; Sn negated so device can subtract)
        Wla[:, H * Cseg + h] = 0.2 * (g[pl] * Wl64[:, pl]).sum(1)
        Wra[:, H * Cseg + h] = 0.2 * (g[pl] * Wr64[:, pl]).sum(1)
        Wla[:, 2 * H * Cseg + H + h] = -5.0 * (g[nl] * Wl64[:, nl]).sum(1)
        Wra[:, 2 * H * Cseg + H + h] = -5.0 * (g[nl] * Wr64[:, nl]).sum(1)
    return Wla.astype(np.float32), Wra.astype(np.float32), pos_lists, neg_lists, g


def _pick_cseg(att, mod):
    att_f = np.asarray(att).reshape(att.shape[0], -1)
    mx = 0
    for h in range(att.shape[0]):
        s = (att_f[h] >= 0).sum()
        mx = max(mx, s, att_f.shape[1] - s)
    c = int(mx)
    while (c % mod) != mod - 1:
        c += 1
    return c


def _packed_maps(pos_lists, neg_lists, Cseg, g, b):
    """invg/b vectors in packed agg space [H*Cseg pos | H*Cseg neg]."""
    H = len(pos_lists)
    npos = H * Cseg
    invg = np.zeros(2 * npos, np.float32)
    bb = np.zeros(2 * npos, np.float32)
    colmap = -np.ones(2 * npos, np.int64)  # packed pos -> orig col
    for h in range(H):
        pl, nl = pos_lists[h], neg_lists[h]
        invg[h * Cseg : h * Cseg + len(pl)] = 1.0 / g[pl]
        bb[h * Cseg : h * Cseg + len(pl)] = b[pl]
        colmap[h * Cseg : h * Cseg + len(pl)] = pl
        invg[npos + h * Cseg : npos + h * Cseg + len(nl)] = 1.0 / g[nl]
        bb[npos + h * Cseg : npos + h * Cseg + len(nl)] = b[nl]
        colmap[npos + h * Cseg : npos + h * Cseg + len(nl)] = nl
    return invg, bb, colmap


def _edge_plan(src, dst):
    """Per-core per-tile chunked edges, chunk counts equalized across cores."""
    counts = np.zeros((NCORES, NT), np.int64)
    per_core = []
    for c in range(NCORES):
        lo = c * NLOC
        sel = (dst >= lo) & (dst < lo + NLOC)
        s, d = src[sel], dst[sel] - lo
        order = np.argsort(d, kind="stable")
        s, d = s[order], d[order]
        tb = np.searchsorted(d, np.arange(0, NT + 1) * P)
        per_core.append((s, d, tb))
        counts[c] = np.maximum(1, (np.diff(tb) + P - 1) // P)
    nct = counts.max(0)  # chunks per tile (same for all cores)
    totch = int(nct.sum())
    choff = np.concatenate([[0], np.cumsum(nct)]).astype(np.int64)

    plans = []
    for c in range(NCORES):
        s, d, tb = per_core[c]
        src_pad = np.zeros((totch, P), np.int64)
        d128 = np.zeros((totch, P), np.int64)
        mask = np.zeros((totch, P), np.float32)
        for t in range(NT):
            a, bnd = tb[t], tb[t + 1]
            ne = bnd - a
            o = choff[t]
            src_pad[o : o + (ne + P - 1) // P].reshape(-1)[:ne] = s[a:bnd]
            d128[o : o + (ne + P - 1) // P].reshape(-1)[:ne] = d[a:bnd] - t * P
            mask[o : o + (ne + P - 1) // P].reshape(-1)[:ne] = 1.0
        # indicator matrices
        ch_i = np.repeat(np.arange(totch), P)
        e_i = np.tile(np.arange(P), totch)
        mm = mask.reshape(-1).astype(bool)
        inddm = np.zeros((totch, P, P), np.float32)
        indt = np.zeros((totch, P, P), np.float32)
        inddm[ch_i[mm], d128.reshape(-1)[mm], e_i[mm]] = 1.0
        indt[ch_i[mm], e_i[mm], d128.reshape(-1)[mm]] = 1.0
        # gather indices, int16 wrap-16 layout replicated to 128 partitions
        flat = src_pad.reshape(-1)
        idx16 = np.zeros((16, totch * 8), np.int16)
        ar = np.arange(totch * P)
        idx16[ar % 16, ar // 16] = flat.astype(np.int16)
        idx = np.tile(idx16, (8, 1))
        maskb = np.where(mask.T == 1.0, 0.0, MASK_NEG).astype(np.float32)  # [P, totch]
        plans.append(dict(inddm=inddm, indt=indt, idx=idx, maskb=maskb))
    return plans, nct, choff, totch


# ---------------------------------------------------------------- bass build


def _build_program(meta):
    import concourse.bass as bass
    import concourse.bacc as bacc
    import concourse.tile as tile
    import concourse.mybir as mybir
    from concourse.masks import make_identity
    from contextlib import ExitStack

    fp = mybir.dt.float32
    i16 = mybir.dt.int16
    AF = mybir.ActivationFunctionType
    OP = mybir.AluOpType
    totch = int(meta["totch"])
    nct = meta["nct"]
    choff = meta["choff"]
    C1, C2 = int(meta["cseg1"]), int(meta["cseg2"])
    F1 = 8 * C1 + 8
    F2 = 2 * C2 + 2
    NP1 = 4 * C1  # pos block width L1
    NP2 = C2
    PK1 = 2 * NP1  # packed agg width L1 (632)
    PK2 = 2 * NP2
    K2 = (PK1 + P - 1) // P  # k-tiles for layer2 contraction (5)
    KP2 = K2 * P  # 640

    nc = bacc.Bacc("TRN2", target_bir_lowering=False, debug=False)

    # ---- I/O
    xT = nc.dram_tensor("xT", [KPAD1, NLOC], fp, kind="ExternalInput")
    Wd = {}
    for nm in ("wl_s1", "wr_s1", "wl_p1", "wr_p1"):
        Wd[nm] = nc.dram_tensor(nm, [KPAD1, F1], fp, kind="ExternalInput")
    for nm in ("wl_s2", "wr_s2", "wl_p2", "wr_p2"):
        Wd[nm] = nc.dram_tensor(nm, [KP2, F2], fp, kind="ExternalInput")
    inddm_d = nc.dram_tensor("inddm", [totch, P, P], fp, kind="ExternalInput")
    indt_d = nc.dram_tensor("indt", [totch, P, P], fp, kind="ExternalInput")
    idx_d = nc.dram_tensor("idx", [P, totch * 8], i16, kind="ExternalInput")
    maskb_d = nc.dram_tensor("maskb", [P, totch], fp, kind="ExternalInput")
    aux = {}
    for nm in ("invg_s1", "b_s1", "invg_p1", "b_p1"):
        aux[nm] = nc.dram_tensor(nm, [P, PK1], fp, kind="ExternalInput")
    for nm in ("invg_s2", "b_s2", "invg_p2", "b_p2"):
        aux[nm] = nc.dram_tensor(nm, [P, PK2], fp, kind="ExternalInput")
    outs = {}
    for nm in ("f1_s", "f2_s", "f1_p", "f2_p"):
        outs[nm] = nc.dram_tensor(nm, [NLOC, PK2], fp, kind="ExternalOutput")

    # ---- internal DRAM
    ag_in = {
        "s1": nc.dram_tensor("agin_s1", [NLOC, F1], fp),
        "p1": nc.dram_tensor("agin_p1", [NLOC, F1], fp),
        "s2": nc.dram_tensor("agin_s2", [NLOC, F2], fp),
        "p2": nc.dram_tensor("agin_p2", [NLOC, F2], fp),
    }
    xl_full = {
        "s1": nc.dram_tensor("xlf_s1", [N, F1], fp, addr_space="Shared"),
        "p1": nc.dram_tensor("xlf_p1", [N, F1], fp, addr_space="Shared"),
        "s2": nc.dram_tensor("xlf_s2", [N, F2], fp, addr_space="Shared"),
        "p2": nc.dram_tensor("xlf_p2", [N, F2], fp, addr_space="Shared"),
    }
    xr_loc = {
        "s1": nc.dram_tensor("xr_s1", [NLOC, F1], fp),
        "p1": nc.dram_tensor("xr_p1", [NLOC, F1], fp),
        "s2": nc.dram_tensor("xr_s2", [NLOC, F2], fp),
        "p2": nc.dram_tensor("xr_p2", [NLOC, F2], fp),
    }
    sT = {
        "s": nc.dram_tensor("sT_s", [KP2, NLOC], fp),
        "p": nc.dram_tensor("sT_p", [KP2, NLOC], fp),
    }

    RG = [list(range(NCORES))]
    GC = 4  # chunks per gather/indicator group

    with tile.TileContext(nc) as tc, ExitStack() as ctx:
        consts = ctx.enter_context(tc.tile_pool(name="consts", bufs=1))
        wres = ctx.enter_context(tc.tile_pool(name="wres", bufs=2))
        w2res = ctx.enter_context(tc.tile_pool(name="w2res", bufs=4))
        slabs = ctx.enter_context(tc.tile_pool(name="slabs", bufs=2))
        ev = ctx.enter_context(tc.tile_pool(name="ev", bufs=3))
        edge = ctx.enter_context(tc.tile_pool(name="edge", bufs=2))
        small = ctx.enter_context(tc.tile_pool(name="small", bufs=3))
        fin = ctx.enter_context(tc.tile_pool(name="fin", bufs=2))
        psbig = ctx.enter_context(tc.tile_pool(name="psbig", bufs=3, space="PSUM"))
        pssm = ctx.enter_context(tc.tile_pool(name="pssm", bufs=2, space="PSUM"))

        ident = consts.tile([P, P], fp)
        make_identity(nc, ident[:])
        eeta = consts.tile([P, 1], fp)
        nc.vector.memset(eeta[:], float(np.exp(ETA)))
        idx_t = consts.tile([P, totch * 8], i16)
        nc.sync.dma_start(out=idx_t[:], in_=idx_d[:])
        maskb_t = consts.tile([P, totch], fp)
        nc.sync.dma_start(out=maskb_t[:], in_=maskb_d[:])
        aux_t = {}
        for nm, d in aux.items():
            a = consts.tile([P, d.shape[1]], fp, tag=nm)
            nc.sync.dma_start(out=a[:], in_=d[:])
            aux_t[nm] = a

        def mrows(t):
            return P if t < NT - 1 else NLOC - (NT - 1) * P

        # ---------------- projection layer 1 (one weight pair, both col halves)
        def proj1(wl_nm, wr_nm, xl_dst, xr_dst):
            CW = F1 // 2  # 320
            for half in range(2):
                co = half * CW
                wl_t = wres.tile([P, K1, CW], fp, tag="w1")
                nc.sync.dma_start(
                    out=wl_t[:],
                    in_=Wd[wl_nm][:, co : co + CW].rearrange("(k p) c -> p k c", p=P),
                )
                wr_t = wres.tile([P, K1, CW], fp, tag="w1")
                nc.sync.dma_start(
                    out=wr_t[:],
                    in_=Wd[wr_nm][:, co : co + CW].rearrange("(k p) c -> p k c", p=P),
                )
                for t in range(NT):
                    mr = mrows(t)
                    slab = slabs.tile([P, K1, P], fp, tag="xtm")
                    nc.sync.dma_start(
                        out=slab[:, :, 0:mr],
                        in_=xT[:, t * P : t * P + mr].rearrange(
                            "(k p) m -> p k m", p=P
                        ),
                    )
                    psl = pssm.tile([P, CW], fp, tag="sm")
                    psr = pssm.tile([P, CW], fp, tag="sm")
                    for k in range(K1):
                        nc.tensor.matmul(
                            psl[0:mr, :], slab[:, k, 0:mr], wl_t[:, k, :],
                            start=(k == 0), stop=(k == K1 - 1),
                        )
                        nc.tensor.matmul(
                            psr[0:mr, :], slab[:, k, 0:mr], wr_t[:, k, :],
                            start=(k == 0), stop=(k == K1 - 1),
                        )
                    for ps, dst in ((psl, xl_dst), (psr, xr_dst)):
                        e = ev.tile([P, CW], fp, tag="ev1")
                        nc.scalar.copy(e[0:mr, :], ps[0:mr, :])
                        nc.sync.dma_start(
                            out=dst[t * P : t * P + mr, co : co + CW], in_=e[0:mr, :]
                        )

        # ---------------- projection layer 2 (from sT, all 4 aug cols at once)
        def proj2(w_nm, src_sT, dst):
            w_t = w2res.tile([P, K2, F2], fp, tag="w2")
            nc.sync.dma_start(
                out=w_t[:], in_=Wd[w_nm][:].rearrange("(k p) c -> p k c", p=P)
            )
            for t in range(NT):
                mr = mrows(t)
                slab = slabs.tile([P, K2, P], fp, tag="stm")
                nc.sync.dma_start(
                    out=slab[:, :, 0:mr],
                    in_=src_sT[:, t * P : t * P + mr].rearrange("(k p) m -> p k m", p=P),
                )
                ps = pssm.tile([P, F2], fp, tag="sm")
                for k in range(K2):
                    nc.tensor.matmul(
                        ps[0:mr, :], slab[:, k, 0:mr], w_t[:, k, :],
                        start=(k == 0), stop=(k == K2 - 1),
                    )
                e = ev.tile([P, F2], fp, tag="ev2")
                nc.scalar.copy(e[0:mr, :], ps[0:mr, :])
                nc.sync.dma_start(out=dst[t * P : t * P + mr, :], in_=e[0:mr, :])

        # ---------------- edge phase (generic over layer)
        def edge_phase(lyr, branch, H, Cseg, F, finalize):
            """lyr in ('1','2'); finalize(t, mr, w_sb) consumes unscale input."""
            key = branch + lyr
            npos = H * Cseg
            xlf = xl_full[key]
            for t in range(NT):
                mr = mrows(t)
                xr_t = edge.tile([P, F], fp, tag="xr" + lyr)
                if mr < P:
                    nc.vector.memset(xr_t[mr // 32 * 32 : P, :], 0.0)
                nc.sync.dma_start(
                    out=xr_t[0:mr, :], in_=xr_loc[key][t * P : t * P + mr, :]
                )
                agg = psbig.tile([P, F], fp, tag="big")
                den = pssm.tile([P, 4], fp, tag="sm")
                ntile = int(nct[t])
                base = int(choff[t])
                for g0 in range(0, ntile, GC):
                    gc = min(GC, ntile - g0)
                    gbuf = edge.tile([P, GC, F], fp, tag="g" + lyr)
                    nc.gpsimd.dma_gather(
                        gbuf[:, 0:gc, :],
                        xlf[:],
                        idx_t[:, (base + g0) * 8 : (base + g0 + gc) * 8],
                        num_idxs=gc * P,
                        num_idxs_reg=gc * P,
                        elem_size=F,
                    )
                    ind_t = edge.tile([P, GC, P], fp, tag="inddm")
                    nc.sync.dma_start(
                        out=ind_t[:, 0:gc, :],
                        in_=inddm_d[base + g0 : base + g0 + gc].rearrange(
                            "j d e -> d j e"
                        ),
                    )
                    indt_t = edge.tile([P, GC, P], fp, tag="indt")
                    nc.sync.dma_start(
                        out=indt_t[:, 0:gc, :],
                        in_=indt_d[base + g0 : base + g0 + gc].rearrange(
                            "j e d -> e j d"
                        ),
                    )
                    for j in range(gc):
                        ch = base + g0 + j
                        first = g0 + j == 0
                        last = g0 + j == ntile - 1
                        tp = psbig.tile([P, F], fp, tag="big")
                        # t = Ind_dm.T @ xr + I @ G   (column splits at 512)
                        for lo in range(0, F, 512):
                            hi = min(lo + 512, F)
                            nc.tensor.matmul(
                                tp[:, lo:hi], ind_t[:, j, :], xr_t[:, lo:hi],
                                start=True, stop=False,
                            )
                            nc.tensor.matmul(
                                tp[:, lo:hi], ident[:], gbuf[:, j, lo:hi],
                                start=False, stop=True,
                            )
                        # tlr segments of (Cseg+1): relu cols + S slot
                        tlr = edge.tile([P, 2 * H * (Cseg + 1)], fp, tag="tlr" + lyr)
                        t0 = tlr[:, 0:1]
                        p0 = tp[:, 0:1]
                        for s in range(2):
                            nc.scalar.activation(
                                bass.AP(
                                    tensor=t0.tensor,
                                    offset=t0.offset + s * H * (Cseg + 1),
                                    ap=[t0.ap[0], [Cseg + 1, H], [1, Cseg]],
                                ),
                                bass.AP(
                                    tensor=p0.tensor,
                                    offset=p0.offset + s * (npos + H),
                                    ap=[p0.ap[0], [Cseg, H], [1, Cseg]],
                                ),
                                AF.Relu,
                                scale=(0.8 if s == 0 else 4.0),
                            )
                        # S slots (psum cols [npos:npos+H] and [2npos+H:2npos+2H])
                        nc.scalar.activation(
                            bass.AP(
                                tensor=t0.tensor,
                                offset=t0.offset + Cseg,
                                ap=[t0.ap[0], [H * (Cseg + 1), 2], [Cseg + 1, H]],
                            ),
                            bass.AP(
                                tensor=p0.tensor,
                                offset=p0.offset + npos,
                                ap=[p0.ap[0], [npos + H, 2], [1, H]],
                            ),
                            AF.Copy,
                        )
                        red = small.tile([P, 2 * H], fp, tag="red")
                        nc.vector.tensor_reduce(
                            red[:],
                            tlr[:].rearrange("p (s h c) -> p s h c", s=2, h=H),
                            axis=mybir.AxisListType.X,
                            op=OP.add,
                        )
                        e_t = small.tile([P, H], fp, tag="e")
                        nc.vector.tensor_tensor(
                            e_t[:], red[:, 0:H], red[:, H : 2 * H], op=OP.subtract
                        )
                        ee = small.tile([P, H], fp, tag="ee")
                        nc.scalar.activation(
                            ee[:], e_t[:], AF.Exp, bias=maskb_t[:, ch : ch + 1]
                        )
                        eeg = edge.tile([P, 2 * npos], fp, tag="eeg" + lyr)
                        eap = ee[:]
                        g0b = gbuf[:, j, 0:1]
                        nc.vector.tensor_tensor(
                            eeg[:].rearrange("p (s h c) -> p s h c", s=2, h=H),
                            bass.AP(
                                tensor=g0b.tensor,
                                offset=g0b.offset,
                                ap=[g0b.ap[0], [npos + H, 2], [Cseg, H], [1, Cseg]],
                            ),
                            bass.AP(
                                tensor=eap.tensor,
                                offset=eap.offset,
                                ap=[eap.ap[0], [0, 2], [1, H], [0, Cseg]],
                            ),
                            op=OP.mult,
                        )
                        for lo in range(0, 2 * npos, 512):
                            hi = min(lo + 512, 2 * npos)
                            nc.tensor.matmul(
                                agg[:, lo:hi], indt_t[:, j, :], eeg[:, lo:hi],
                                start=first, stop=last,
                            )
                        nc.tensor.matmul(
                            den[:, 0:H], indt_t[:, j, :], ee[:], start=first, stop=last
                        )
                # ---- finalize tile: w = (agg/den) * invg + b
                rec = small.tile([P, H], fp, tag="rec")
                nc.vector.reciprocal(rec[0:mr, :], den[0:mr, 0:H])
                w_sb = fin.tile([P, 2 * npos], fp, tag="w" + lyr)
                if mr < P:
                    nc.vector.memset(w_sb[mr // 32 * 32 : P, :], 0.0)
                rap = rec[0:mr, :]
                nc.vector.tensor_tensor(
                    w_sb[0:mr, :].rearrange("p (s h c) -> p s h c", s=2, h=H),
                    agg[0:mr, 0 : 2 * npos].rearrange("p (s h c) -> p s h c", s=2, h=H),
                    bass.AP(
                        tensor=rap.tensor,
                        offset=rap.offset,
                        ap=[rap.ap[0], [0, 2], [1, H], [0, Cseg]],
                    ),
                    op=OP.mult,
                )
                ig = aux_t["invg_" + key]
                bt = aux_t["b_" + key]
                nc.vector.tensor_tensor(
                    w_sb[0:mr, :], w_sb[0:mr, :], ig[0:mr, :], op=OP.mult
                )
                nc.vector.tensor_tensor(
                    w_sb[0:mr, :], w_sb[0:mr, :], bt[0:mr, :], op=OP.add
                )
                finalize(t, mr, w_sb)

        # finalize for layer1: (relu) + transpose into sT
        def fin1(branch, do_relu):
            dst = sT[branch]

            def f(t, mr, w_sb):
                if do_relu:
                    nc.vector.tensor_scalar_max(w_sb[0:mr, :], w_sb[0:mr, :], 0.0)
                stb = fin.tile([P, K2, P], fp, tag="stb")
                if PK1 < K2 * P:
                    # zero tail partitions of last block (32-aligned start)
                    z0 = (PK1 - (K2 - 1) * P) // 32 * 32
                    nc.vector.memset(stb[z0:P, K2 - 1, :], 0.0)
                for blk in range(K2):
                    lo = blk * P
                    hi = min(lo + P, PK1)
                    bw = hi - lo
                    tps = pssm.tile([P, P], fp, tag="sm")
                    nc.tensor.transpose(tps[0:bw, :], w_sb[:, lo:hi], ident[:])
                    nc.scalar.copy(stb[0:bw, blk, :], tps[0:bw, :])
                nc.sync.dma_start(
                    out=dst[:, t * P : t * P + mr].rearrange("(k p) m -> p k m", p=P),
                    in_=stb[:, :, 0:mr],
                )

            return f

        # finalize for layer2: f1 = w, f2 = softplus(w) + eta
        def fin2(branch):
            def f(t, mr, w_sb):
                f1 = fin.tile([P, PK2], fp, tag="f1")
                nc.vector.tensor_copy(f1[0:mr, :], w_sb[0:mr, :])
                nc.sync.dma_start(
                    out=outs["f1_" + branch][t * P : t * P + mr, :], in_=f1[0:mr, :]
                )
                ex = fin.tile([P, PK2], fp, tag="ex")
                nc.scalar.activation(ex[0:mr, :], w_sb[0:mr, :], AF.Exp)
                f2 = fin.tile([P, PK2], fp, tag="f2")
                sc = float(np.exp(ETA))
                nc.scalar.activation(
                    f2[0:mr, :], ex[0:mr, :], AF.Ln, bias=eeta[0:mr, :], scale=sc
                )
                nc.sync.dma_start(
                    out=outs["f2_" + branch][t * P : t * P + mr, :], in_=f2[0:mr, :]
                )

            return f

        def allgather(key):
            nc.gpsimd.collective_compute(
                "AllGather",
                mybir.AluOpType.bypass,
                replica_groups=RG,
                ins=[ag_in[key][:]],
                outs=[xl_full[key][:]],
            )

        # ---------------- program
        proj1("wl_s1", "wr_s1", ag_in["s1"], xr_loc["s1"])
        allgather("s1")
        proj1("wl_p1", "wr_p1", ag_in["p1"], xr_loc["p1"])
        allgather("p1")
        edge_phase("1", "s", 4, C1, F1, fin1("s", True))
        proj2("wl_s2", sT["s"], ag_in["s2"])
        proj2("wr_s2", sT["s"], xr_loc["s2"])
        allgather("s2")
        edge_phase("1", "p", 4, C1, F1, fin1("p", False))
        proj2("wl_p2", sT["p"], ag_in["p2"])
        proj2("wr_p2", sT["p"], xr_loc["p2"])
        allgather("p2")
        edge_phase("2", "s", 1, C2, F2, fin2("s"))
        edge_phase("2", "p", 1, C2, F2, fin2("p"))

    nc.compile()
    return nc


# ---------------------------------------------------------------- entry point

_CACHE = {}
LAST_RESULT = None
LAST_EXEC_NS = None


def _run_pjrt_timed(nc, in_maps, n_cores, reps):
    """Mirror bass2jax.run_bass_via_pjrt's multi-core path, but pre-stage
    inputs on device and time pure executions."""
    import time
    import jax
    import numpy as np
    from jax.sharding import Mesh, PartitionSpec
    from jax.experimental.shard_map import shard_map
    import concourse.mybir as mybir
    from concourse import bass2jax

    bass2jax.install_neuronx_cc_hook()
    partition_name = (
        nc.partition_id_tensor.name if nc.partition_id_tensor else None
    )
    in_names, out_names, out_avals, zero_outs = [], [], [], []
    for alloc in nc.m.functions[0].allocations:
        if not isinstance(alloc, mybir.MemoryLocationSet):
            continue
        name = alloc.memorylocations[0].name
        if alloc.kind == "ExternalInput":
            if name != partition_name:
                in_names.append(name)
        elif alloc.kind == "ExternalOutput":
            out_names.append(name)
            shape = tuple(alloc.tensor_shape)
            dtype = mybir.dt.np(alloc.dtype)
            out_avals.append(jax.core.ShapedArray(shape, dtype))
            zero_outs.append(np.zeros(shape, dtype))
    n_params = len(in_names)
    all_names = list(in_names) + out_names
    if partition_name is not None:
        all_names.append(partition_name)

    def _body(*args):
        operands = list(args)
        if partition_name is not None:
            operands.append(bass2jax.partition_id_tensor())
        return tuple(
            bass2jax._bass_exec_p.bind(
                *operands,
                out_avals=tuple(out_avals),
                in_names=tuple(all_names),
                out_names=tuple(out_names),
                lowering_input_output_aliases=(),
                sim_require_finite=True,
                sim_require_nnan=True,
                nc=nc,
            )
        )

    devices = jax.devices()[:n_cores]
    mesh = Mesh(np.asarray(devices), ("core",))
    nin = n_params + len(zero_outs)
    sharded = jax.jit(
        shard_map(
            _body,
            mesh=mesh,
            in_specs=(PartitionSpec("core"),) * nin,
            out_specs=(PartitionSpec("core"),) * len(out_names),
            check_rep=False,
        ),
        keep_unused=True,
    )
    concat_in = [
        np.concatenate([np.asarray(in_maps[c][nm]) for c in range(n_cores)], 0)
        for nm in in_names
    ]
    concat_zeros = [
        np.zeros((n_cores * z.shape[0], *z.shape[1:]), z.dtype) for z in zero_outs
    ]
    sh = jax.sharding.NamedSharding(mesh, PartitionSpec("core"))
    dev_in = [jax.device_put(a, sh) for a in concat_in + concat_zeros]
    out = sharded(*dev_in)
    jax.block_until_ready(out)
    times = []
    for _ in range(reps):
        t0 = time.perf_counter()
        out = sharded(*dev_in)
        jax.block_until_ready(out)
        times.append(time.perf_counter() - t0)
    global LAST_EXEC_NS
    LAST_EXEC_NS = int(min(times) * 1e9)
    results = [
        {
            nm: np.asarray(out[i]).reshape(n_cores, *out_avals[i].shape)[c]
            for i, nm in enumerate(out_names)
        }
        for c in range(n_cores)
    ]
    return results


def kernel(x, edge_index, Wl_s1, Wr_s1, att_s1, b_s1, Wl_s2, Wr_s2, att_s2, b_s2,
           Wl_p1, Wr_p1, att_p1, b_p1, Wl_p2, Wr_p2, att_p2, b_p2):
    from concourse.bass_utils import run_bass_kernel_spmd

    x = np.asarray(x, np.float32)
    ei = np.asarray(edge_index, np.int64)
    src = np.concatenate([ei[0], np.arange(N, dtype=np.int64)])
    dst = np.concatenate([ei[1], np.arange(N, dtype=np.int64)])

    C1 = _pick_cseg(np.asarray(att_s1), 8)
    C1 = max(C1, _pick_cseg(np.asarray(att_p1), 8))
    C2 = _pick_cseg(np.asarray(att_s2), 32)
    C2 = max(C2, _pick_cseg(np.asarray(att_p2), 32))
    F1 = 8 * C1 + 8
    F2 = 2 * C2 + 2
    PK1, PK2 = 8 * C1, 2 * C2
    K2 = (PK1 + P - 1) // P
    KP2 = K2 * P

    # ---- weights
    wdat = {}
    maps = {}
    for key, (Wl, Wr, att, b, Cs) in {
        "s1": (Wl_s1, Wr_s1, att_s1, b_s1, C1),
        "p1": (Wl_p1, Wr_p1, att_p1, b_p1, C1),
    }.items():
        Wla, Wra, pl, nl, g = _fold_weights(
            np.asarray(Wl), np.asarray(Wr), np.asarray(att), Cs
        )
        invg, bb, colmap = _packed_maps(pl, nl, Cs, g, np.asarray(b))
        pad = np.zeros((KPAD1 - IN_DIM, F1), np.float32)
        wdat["wl_" + key] = np.concatenate([Wla, pad], 0)
        wdat["wr_" + key] = np.concatenate([Wra, pad], 0)
        maps[key] = (invg, bb, colmap)

    for key, (Wl, Wr, att, b, Cs, l1key) in {
        "s2": (Wl_s2, Wr_s2, att_s2, b_s2, C2, "s1"),
        "p2": (Wl_p2, Wr_p2, att_p2, b_p2, C2, "p1"),
    }.items():
        colmap1 = maps[l1key][2]
        # remap rows: W2 row for packed L1 position j = W2[colmap1[j]]
        def remap(W):
            W = np.asarray(W, np.float32)
            Wp = np.zeros((KP2, W.shape[1]), np.float32)
            ok = colmap1 >= 0
            Wp[: len(colmap1)][ok] = W[colmap1[ok]]
            return Wp

        Wla, Wra, pl, nl, g = _fold_weights(remap(Wl), remap(Wr), np.asarray(att), Cs)
        invg, bb, colmap = _packed_maps(pl, nl, Cs, g, np.asarray(b))
        wdat["wl_" + key] = Wla
        wdat["wr_" + key] = Wra
        maps[key] = (invg, bb, colmap)

    auxdat = {}
    for key in ("s1", "p1", "s2", "p2"):
        invg, bb, _ = maps[key]
        auxdat["invg_" + key] = np.tile(invg, (P, 1))
        auxdat["b_" + key] = np.tile(bb, (P, 1))

    # ---- x transposed + padded
    xTfull = np.zeros((KPAD1, N), np.float32)
    xTfull[:IN_DIM] = x.T

    # ---- edges
    plans, nct, choff, totch = _edge_plan(src, dst)

    meta = dict(cseg1=C1, cseg2=C2, totch=totch, nct=nct, choff=choff)
    ck = (C1, C2, totch, tuple(nct))
    if ck not in _CACHE:
        _CACHE[ck] = _build_program(meta)
    nc = _CACHE[ck]

    in_maps = []
    for c in range(NCORES):
        m = dict(
            xT=np.ascontiguousarray(xTfull[:, c * NLOC : (c + 1) * NLOC]),
            inddm=plans[c]["inddm"],
            indt=plans[c]["indt"],
            idx=plans[c]["idx"],
            maskb=plans[c]["maskb"],
        )
        m.update(wdat)
        m.update(auxdat)
        in_maps.append(m)

    import os

    reps = int(os.environ.get("KERNEL_TIME_REPS", "0"))
    if reps > 0:
        results = _run_pjrt_timed(nc, in_maps, NCORES, reps)

        class _R:
            pass

        res = _R()
        res.results = results
        res.exec_time_ns = LAST_EXEC_NS
    else:
        res = run_bass_kernel_spmd(nc, in_maps, core_ids=list(range(NCORES)))
    global LAST_RESULT
    LAST_RESULT = res

    # ---- assemble output
    def gather_out(nm):
        return np.concatenate([res.results[c][nm] for c in range(NCORES)], 0)

    def unpack(f1, f2, key, ncols):
        colmap = maps[key][2]
        val = np.zeros((N, ncols), np.float32)
        sp = np.zeros((N, ncols), np.float32)
        ok = colmap >= 0
        val[:, colmap[ok]] = f1[:, np.nonzero(ok)[0]]
        sp[:, colmap[ok]] = f2[:, np.nonzero(ok)[0]]
        return val, sp

    s_val, s_sp = unpack(gather_out("f1_s"), gather_out("f2_s"), "s2", 2 * S_DIM)
    p_val, p_sp = unpack(gather_out("f1_p"), gather_out("f2_p"), "p2", 2 * P_DIM)
    mu = np.concatenate([s_val[:, :S_DIM], p_val[:, :P_DIM]], 1)
    sig = np.concatenate([s_sp[:, S_DIM:], p_sp[:, P_DIM:]], 1)
    return mu, sig
